# revision 2
# baseline (speedup 1.0000x reference)
"""DeepSeekV3-style MoE on 8 Trainium2 NeuronCores (Bass/Tile) — sparse dispatch.

Strategy (expert-parallel, top-4/16 sparse):
- Host computes routing exactly (float64 sigmoid-gating, group top-2, top-4)
  and builds, per core, the gathered hidden columns for its 2 local experts
  (capacity CAP slots each, padded with gating-weight-0 duplicates of token 0).
- Device per core:
    Phase S: shared expert, F-sharded (256 of 2048), streamed over all T
             tokens; down-projection emitted token-major [tok, H] and written
             densely into 4 [T, 512] DRAM partial buffers (one per H-slice).
    Phase R: for each of 2 local experts: gate/up/silu on the CAP gathered
             tokens, then token-major down-projection scaled by the gating
             weight, dma_scatter_add'ed into the partial buffers (one call
             per expert per H-slice; indices are unique within a call).
    4 bf16 ReduceScatters (one per H-slice) sum partials across cores; each
    core ends with its 512 owned token rows, converted to fp32 and output.
- All matmuls bf16 (full PE rate); accumulation fp32 in PSUM.

Self-contained: shapes hardcoded for the nn_DeepSeekV3StyleMoE problem.
"""

import numpy as np

import concourse.bass as bass
import concourse.mybir as mybir
import concourse.tile as tile
from concourse import bacc
from concourse.bass_utils import run_bass_kernel_spmd

F32 = mybir.dt.float32
BF16 = mybir.dt.bfloat16
I16 = mybir.dt.int16

# problem dims
E = 16          # experts
EL = 2          # local experts per core
NCORES = 8
TOPK = 4
G = 4           # routing groups
EPG = 4         # experts per group
SCALE = 2.5
H = 2048
F = 1024        # moe intermediate
FSH = 2048      # shared intermediate (full)
FSHL = FSH // NCORES  # 256 per core
B, S = 2, 2048
T = B * S       # 4096 tokens
P = 128
KH = H // P     # 16
KF = F // P     # 8
KFS = FSHL // P  # 2
NHS = 4         # H-slices of 512
HS = H // NHS   # 512
TCS = 512       # shared-phase token chunk
NCS = T // TCS  # 8 chunks
TOWN = T // NCORES  # 512 owned tokens per core

_CACHED = {}


def _build(CAP):
    CAPB = CAP // P           # slot blocks per expert
    # gate/up psum column chunks (PSUM bank holds 512 fp32)
    TSL = []
    r = CAP
    while r > 0:
        TSL.append(min(512, r))
        r -= min(512, r)

    nc = bacc.Bacc("TRN2", target_bir_lowering=False, debug=False, num_devices=NCORES)

    hidd_in = nc.dram_tensor("hidd", [H, T], BF16, kind="ExternalInput")
    hidg_in = nc.dram_tensor("hidg", [EL, KH, P, CAP], BF16, kind="ExternalInput")
    gw_in = nc.dram_tensor("gw", [EL, KF, P, KH, P], BF16, kind="ExternalInput")
    uw_in = nc.dram_tensor("uw", [EL, KF, P, KH, P], BF16, kind="ExternalInput")
    dwt_in = nc.dram_tensor("dwt", [EL, KF, P, H], BF16, kind="ExternalInput")
    shg_in = nc.dram_tensor("shg", [KFS, P, KH, P], BF16, kind="ExternalInput")
    shu_in = nc.dram_tensor("shu", [KFS, P, KH, P], BF16, kind="ExternalInput")
    shdt_in = nc.dram_tensor("shdt", [KFS, P, H], BF16, kind="ExternalInput")
    gat_in = nc.dram_tensor("gat", [EL, P, CAPB], F32, kind="ExternalInput")
    sidx_in = nc.dram_tensor("sidx", [EL, P, CAP // 16], I16, kind="ExternalInput")
    out_ext = nc.dram_tensor("out", [TOWN, H], F32, kind="ExternalOutput")

    with tile.TileContext(nc) as tc:
        with (
            tc.tile_pool(name="shconst", bufs=1) as shconst,
            tc.tile_pool(name="hidsp", bufs=1) as hidsp,
            tc.tile_pool(name="hidgp", bufs=1) as hidgp,
            tc.tile_pool(name="wgt", bufs=2) as wgt,
            tc.tile_pool(name="dwp", bufs=1) as dwp,
            tc.tile_pool(name="silp", bufs=2) as silp,
            tc.tile_pool(name="actp", bufs=1) as actp,
            tc.tile_pool(name="stgp", bufs=1) as stgp,
            tc.tile_pool(name="outp", bufs=4) as outp,
            tc.tile_pool(name="smallp", bufs=1) as smallp,
            tc.tile_pool(name="ps_gu", bufs=2, space="PSUM") as ps_gu,
            tc.tile_pool(name="ps_d", bufs=4, space="PSUM") as ps_d,
            tc.tile_pool(name="dram", bufs=1, space="DRAM") as dram,
        ):
            # DRAM partials, one per 512-wide H-slice
            parts = [dram.tile([T, HS], BF16, name=f"part{h}", tag=f"part{h}")
                     for h in range(NHS)]
            rsouts = [dram.tile([TOWN, HS], BF16, name=f"rsout{h}", tag=f"rsout{h}")
                      for h in range(NHS)]

            # small per-expert constants
            gats = []
            sidxs = []
            for e in range(EL):
                g = smallp.tile([P, CAPB], F32, tag=f"gat{e}")
                nc.sync.dma_start(g[:], gat_in.ap()[e])
                gats.append(g)
                si = smallp.tile([P, CAP // 16], I16, tag=f"sidx{e}")
                nc.sync.dma_start(si[:], sidx_in.ap()[e])
                sidxs.append(si)

            # shared expert weights (resident, small)
            shgs, shus, shds = [], [], []
            for f in range(KFS):
                g = shconst.tile([P, KH * P], BF16, tag=f"shg{f}")
                nc.sync.dma_start(g[:], shg_in.ap()[f].rearrange("p k j -> p (k j)"))
                shgs.append(g)
                u = shconst.tile([P, KH * P], BF16, tag=f"shu{f}")
                nc.sync.dma_start(u[:], shu_in.ap()[f].rearrange("p k j -> p (k j)"))
                shus.append(u)
                d = shconst.tile([P, H], BF16, tag=f"shd{f}")
                nc.sync.dma_start(d[:], shdt_in.ap()[f])
                shds.append(d)

            # ---------------- Phase S: shared expert over all tokens ----------
            for c in range(NCS):
                sl_t = slice(c * TCS, (c + 1) * TCS)
                hidt = []
                for k in range(KH):
                    t_ = hidsp.tile([P, TCS], BF16, tag=f"h{k}", name=f"hs_{c}_{k}")
                    nc.sync.dma_start(t_[:], hidd_in.ap()[k * P:(k + 1) * P, sl_t])
                    hidt.append(t_)
                sacts = []
                for f in range(KFS):
                    gp = ps_gu.tile([P, TCS], F32, tag="gp", name=f"sgp_{c}_{f}")
                    up = ps_gu.tile([P, TCS], F32, tag="up", name=f"sup_{c}_{f}")
                    for k in range(KH):
                        nc.tensor.matmul(
                            gp[:], shgs[f][:, k * P:(k + 1) * P], hidt[k][:],
                            start=(k == 0), stop=(k == KH - 1),
                        )
                    for k in range(KH):
                        nc.tensor.matmul(
                            up[:], shus[f][:, k * P:(k + 1) * P], hidt[k][:],
                            start=(k == 0), stop=(k == KH - 1),
                        )
                    sil = silp.tile([P, TCS], F32, tag="sil", name=f"ssil_{c}_{f}")
                    nc.scalar.activation(sil[:], gp[:], mybir.ActivationFunctionType.Silu)
                    at = actp.tile([P, TCS], BF16, tag=f"sact{f}", name=f"sact_{c}_{f}")
                    nc.vector.tensor_tensor(at[:], sil[:], up[:], mybir.AluOpType.mult)
                    sacts.append(at)
                # token-major down: out[tok, H]
                for tb in range(TCS // P):
                    dps = []
                    for h in range(NHS):
                        dp = ps_d.tile([P, HS], F32, tag="dp", name=f"sdp_{c}_{tb}_{h}")
                        dps.append(dp)
                    for f in range(KFS):
                        for h in range(NHS):
                            nc.tensor.matmul(
                                dps[h][:],
                                sacts[f][:, tb * P:(tb + 1) * P],
                                shds[f][:, h * HS:(h + 1) * HS],
                                start=(f == 0), stop=(f == KFS - 1),
                            )
                    for h in range(NHS):
                        ob = outp.tile([P, HS], BF16, tag="sob", name=f"sob_{c}_{tb}_{h}")
                        nc.vector.tensor_copy(ob[:], dps[h][:])
                        nc.sync.dma_start(
                            parts[h][c * TCS + tb * P: c * TCS + (tb + 1) * P, :],
                            ob[:],
                        )

            # ---------------- Phase R: routed experts -------------------------
            def load_hidg(e):
                tiles = []
                for k in range(KH):
                    t_ = hidgp.tile([P, CAP], BF16, tag=f"hg{k}", name=f"hg_{e}_{k}")
                    nc.sync.dma_start(t_[:], hidg_in.ap()[e, k])
                    tiles.append(t_)
                return tiles

            def load_dwt(e):
                tiles = []
                for kf in range(KF):
                    t_ = dwp.tile([P, H], BF16, tag=f"dw{kf}", name=f"dwt_{e}_{kf}")
                    nc.sync.dma_start(t_[:], dwt_in.ap()[e, kf])
                    tiles.append(t_)
                return tiles

            hidg_e = load_hidg(0)
            dwt_e = load_dwt(0)

            for e in range(EL):
                # ---- gate/up/act over CAP gathered tokens ----
                acts = []
                for f in range(KF):
                    gt = wgt.tile([P, KH * P], BF16, tag="gt", name=f"gt_{e}_{f}")
                    nc.sync.dma_start(gt[:], gw_in.ap()[e, f].rearrange("p k j -> p (k j)"))
                    ut = wgt.tile([P, KH * P], BF16, tag="ut", name=f"ut_{e}_{f}")
                    nc.sync.dma_start(ut[:], uw_in.ap()[e, f].rearrange("p k j -> p (k j)"))
                    at = actp.tile([P, CAP], BF16, tag=f"act{f}", name=f"act_{e}_{f}")
                    col = 0
                    for ts in TSL:
                        sl_s = slice(col, col + ts)
                        gp = ps_gu.tile([P, ts], F32, tag="gp", name=f"gp_{e}_{f}_{col}")
                        up = ps_gu.tile([P, ts], F32, tag="up", name=f"up_{e}_{f}_{col}")
                        for k in range(KH):
                            nc.tensor.matmul(
                                gp[:], gt[:, k * P:(k + 1) * P], hidg_e[k][:, sl_s],
                                start=(k == 0), stop=(k == KH - 1),
                            )
                        for k in range(KH):
                            nc.tensor.matmul(
                                up[:], ut[:, k * P:(k + 1) * P], hidg_e[k][:, sl_s],
                                start=(k == 0), stop=(k == KH - 1),
                            )
                        sil = silp.tile([P, ts], F32, tag="sil", name=f"sil_{e}_{f}_{col}")
                        nc.scalar.activation(sil[:], gp[:], mybir.ActivationFunctionType.Silu)
                        nc.vector.tensor_tensor(at[:, sl_s], sil[:], up[:], mybir.AluOpType.mult)
                        col += ts
                    acts.append(at)

                # prefetch next expert's gathered hidden (reuses same buffers,
                # waits until this expert's gate/up has read them)
                if e + 1 < EL:
                    hidg_next = load_hidg(e + 1)

                # ---- token-major down, scaled by gating, scatter-add ----
                stgs = [stgp.tile([P, CAPB * HS], BF16, tag=f"stg{h}", name=f"stg_{e}_{h}")
                        for h in range(NHS)]
                if e == 0:
                    # b-outer (efficient LDWEIGHTS amortization)
                    for b in range(CAPB):
                        dps = [ps_d.tile([P, HS], F32, tag="dp", name=f"dp_{e}_{b}_{h}")
                               for h in range(NHS)]
                        for kf in range(KF):
                            for h in range(NHS):
                                nc.tensor.matmul(
                                    dps[h][:],
                                    acts[kf][:, b * P:(b + 1) * P],
                                    dwt_e[kf][:, h * HS:(h + 1) * HS],
                                    start=(kf == 0), stop=(kf == KF - 1),
                                )
                        for h in range(NHS):
                            nc.vector.tensor_tensor(
                                stgs[h][:, b * HS:(b + 1) * HS], dps[h][:],
                                gats[e][:, b:b + 1].to_broadcast([P, HS]),
                                mybir.AluOpType.mult,
                            )
                    for h in range(NHS):
                        nc.gpsimd.dma_scatter_add(
                            parts[h][:, :],
                            stgs[h][:].rearrange("p (b j) -> p b j", j=HS),
                            sidxs[e][:],
                            num_idxs=CAP,
                            num_idxs_reg=CAP,
                            elem_size=HS,
                        )
                else:
                    # hs-outer: finish H-slices early so ReduceScatters overlap
                    for h in range(NHS):
                        for b in range(CAPB):
                            dp = ps_d.tile([P, HS], F32, tag="dp", name=f"dp_{e}_{b}_{h}")
                            for kf in range(KF):
                                nc.tensor.matmul(
                                    dp[:],
                                    acts[kf][:, b * P:(b + 1) * P],
                                    dwt_e[kf][:, h * HS:(h + 1) * HS],
                                    start=(kf == 0), stop=(kf == KF - 1),
                                )
                            nc.vector.tensor_tensor(
                                stgs[h][:, b * HS:(b + 1) * HS], dp[:],
                                gats[e][:, b:b + 1].to_broadcast([P, HS]),
                                mybir.AluOpType.mult,
                            )
                        nc.gpsimd.dma_scatter_add(
                            parts[h][:, :],
                            stgs[h][:].rearrange("p (b j) -> p b j", j=HS),
                            sidxs[e][:],
                            num_idxs=CAP,
                            num_idxs_reg=CAP,
                            elem_size=HS,
                        )
                        # ReduceScatter this H-slice (both experts done)
                        nc.gpsimd.collective_compute(
                            "ReduceScatter",
                            mybir.AluOpType.add,
                            replica_groups=[list(range(NCORES))],
                            ins=[parts[h].opt()],
                            outs=[rsouts[h].opt()],
                        )
                        # convert owned rows to fp32 and write out
                        for tb in range(TOWN // P):
                            cb = outp.tile([P, HS], BF16, tag="cb", name=f"cb_{h}_{tb}")
                            nc.sync.dma_start(cb[:], rsouts[h][tb * P:(tb + 1) * P, :])
                            cf = outp.tile([P, HS], F32, tag="cf", name=f"cf_{h}_{tb}")
                            nc.vector.tensor_copy(cf[:], cb[:])
                            nc.sync.dma_start(
                                out_ext.ap()[tb * P:(tb + 1) * P, h * HS:(h + 1) * HS],
                                cf[:],
                            )

                if e + 1 < EL:
                    hidg_e = hidg_next
                    dwt_e = load_dwt(e + 1)

    nc.compile()
    return nc


def _route_host(hidden, gate_weight, bias):
    """Exact replication of the reference routing in float64."""
    logits = hidden.astype(np.float64) @ gate_weight.astype(np.float64).T
    scores = 1.0 / (1.0 + np.exp(-logits))                  # [T, E]
    sfc = scores + bias.astype(np.float64)[None, :]
    Tn = scores.shape[0]
    grp = sfc.reshape(Tn, G, EPG)
    top2 = np.sort(grp, axis=-1)[:, :, -2:].sum(-1)         # [T, G]
    gidx = np.argsort(-top2, axis=1, kind="stable")[:, :2]
    gmask = np.zeros((Tn, G), bool)
    np.put_along_axis(gmask, gidx, True, 1)
    smask = np.repeat(gmask, EPG, axis=1)
    masked = np.where(smask, sfc, -np.inf)
    tidx = np.argsort(-masked, axis=1, kind="stable")[:, :TOPK]   # [T, K]
    tw = np.take_along_axis(scores, tidx, axis=1)           # raw sigmoid scores
    tw = tw / (tw.sum(-1, keepdims=True) + 1e-20) * SCALE
    return tw.astype(np.float32), tidx


def _prep_core_inputs(core, hidbf, gate_w, up_w, down_w, sh_gate_w, sh_up_w,
                      sh_down_w, slot_tok, slot_gat, CAP):
    import ml_dtypes
    bf = ml_dtypes.bfloat16
    e0 = 2 * core

    def tile_kxm(w):  # w [F', H] -> [KF', P, KH, P]; lhsT[p(H), j(F)] per (f,k)
        Fp = w.shape[0]
        return np.ascontiguousarray(w.reshape(Fp // P, P, KH, P).transpose(0, 3, 2, 1))

    gw = np.stack([tile_kxm(gate_w[e0 + e]) for e in range(EL)])
    uw = np.stack([tile_kxm(up_w[e0 + e]) for e in range(EL)])
    # down token-major: dwt[e, kf, p, h] = down_w[e][h, kf*128+p]
    dwt = np.stack([np.ascontiguousarray(down_w[e0 + e].T.reshape(KF, P, H))
                    for e in range(EL)])

    sl = slice(core * FSHL, (core + 1) * FSHL)
    shg = tile_kxm(sh_gate_w[sl])
    shu = tile_kxm(sh_up_w[sl])
    shdt = np.ascontiguousarray(sh_down_w[:, sl].T.reshape(KFS, P, H))

    # gathered hidden, gating, scatter indices for the 2 local experts
    hidg = np.empty((EL, KH, P, CAP), dtype=bf)
    gat = np.zeros((EL, P, CAP // P), dtype=np.float32)
    sidx = np.zeros((EL, P, CAP // 16), dtype=np.int16)
    for e in range(EL):
        toks = slot_tok[e0 + e]
        gats = slot_gat[e0 + e]
        hidg[e] = hidbf[:, toks].reshape(KH, P, CAP)
        gat[e] = gats.reshape(CAP // P, P).T
        sidx[e] = np.tile(toks.astype(np.int16).reshape(CAP // 16, 16).T, (8, 1))

    return {
        "hidd": hidbf, "hidg": hidg,
        "gw": gw.astype(bf), "uw": uw.astype(bf), "dwt": dwt.astype(bf),
        "shg": shg.astype(bf), "shu": shu.astype(bf), "shdt": shdt.astype(bf),
        "gat": gat, "sidx": sidx,
    }


def kernel(hidden_states, gate_weight, e_score_correction_bias,
           gate_w, up_w, down_w, sh_gate_w, sh_up_w, sh_down_w):
    import ml_dtypes
    bf = ml_dtypes.bfloat16
    hidden_states = np.asarray(hidden_states, dtype=np.float32)
    gate_weight = np.asarray(gate_weight, dtype=np.float32)
    bias = np.asarray(e_score_correction_bias, dtype=np.float32)
    gate_w = np.asarray(gate_w, dtype=np.float32)
    up_w = np.asarray(up_w, dtype=np.float32)
    down_w = np.asarray(down_w, dtype=np.float32)
    sh_gate_w = np.asarray(sh_gate_w, dtype=np.float32)
    sh_up_w = np.asarray(sh_up_w, dtype=np.float32)
    sh_down_w = np.asarray(sh_down_w, dtype=np.float32)

    hidden = hidden_states.reshape(T, H)
    tw, tidx = _route_host(hidden, gate_weight, bias)

    # per-expert slot lists (sorted, padded with token 0 / gating 0)
    counts = np.bincount(tidx.ravel(), minlength=E)
    CAP = max(1152, int(np.ceil(counts.max() / P)) * P)
    slot_tok, slot_gat = [], []
    for e in range(E):
        rows, cols = np.nonzero(tidx == e)
        order = np.argsort(rows, kind="stable")
        toks = rows[order].astype(np.int64)
        gats = tw[rows[order], cols[order]]
        pad = CAP - len(toks)
        # pad slots must carry UNIQUE indices within the scatter call (gating 0
        # makes their contribution exactly +0.0): duplicate rows in one
        # dma_scatter_add race on the read-modify-write and can drop real adds.
        unused = np.setdiff1d(np.arange(T, dtype=np.int64), toks)[:pad]
        assert len(unused) == pad, (len(unused), pad)
        slot_tok.append(np.concatenate([toks, unused]))
        slot_gat.append(np.concatenate([gats, np.zeros(pad, np.float32)]).astype(np.float32))

    if ("nc", CAP) not in _CACHED:
        _CACHED[("nc", CAP)] = _build(CAP)
    nc = _CACHED[("nc", CAP)]

    hidbf = np.ascontiguousarray(hidden.T).astype(bf)   # [H, T]
    in_maps = [
        _prep_core_inputs(c, hidbf, gate_w, up_w, down_w, sh_gate_w, sh_up_w,
                          sh_down_w, slot_tok, slot_gat, CAP)
        for c in range(NCORES)
    ]
    res = run_bass_kernel_spmd(nc, in_maps, core_ids=list(range(NCORES)))
    _CACHED["last_res"] = res
    out = np.concatenate([res.results[c]["out"] for c in range(NCORES)], axis=0)
    return out.reshape(B, S, H).astype(np.float32)


# revision 3
# speedup vs baseline: 1.0168x; 1.0168x over previous
"""DeepSeekV3-style MoE on 8 Trainium2 NeuronCores (Bass/Tile) — sparse dispatch.

Strategy (expert-parallel, top-4/16 sparse):
- Host computes routing exactly (float64 sigmoid-gating, group top-2, top-4)
  and builds, per core, the gathered hidden columns for its 2 local experts
  (capacity CAP slots each, padded with gating-weight-0 duplicates of token 0).
- Device per core:
    Phase S: shared expert, F-sharded (256 of 2048), streamed over all T
             tokens; down-projection emitted token-major [tok, H] and written
             densely into 4 [T, 512] DRAM partial buffers (one per H-slice).
    Phase R: for each of 2 local experts: gate/up/silu on the CAP gathered
             tokens, then token-major down-projection scaled by the gating
             weight, dma_scatter_add'ed into the partial buffers (one call
             per expert per H-slice; indices are unique within a call).
    4 bf16 ReduceScatters (one per H-slice) sum partials across cores; each
    core ends with its 512 owned token rows, converted to fp32 and output.
- All matmuls bf16 (full PE rate); accumulation fp32 in PSUM.

Self-contained: shapes hardcoded for the nn_DeepSeekV3StyleMoE problem.
"""

import numpy as np

import concourse.bass as bass
import concourse.mybir as mybir
import concourse.tile as tile
from concourse import bacc
from concourse.bass_utils import run_bass_kernel_spmd

F32 = mybir.dt.float32
BF16 = mybir.dt.bfloat16
I16 = mybir.dt.int16

# problem dims
E = 16          # experts
EL = 2          # local experts per core
NCORES = 8
TOPK = 4
G = 4           # routing groups
EPG = 4         # experts per group
SCALE = 2.5
H = 2048
F = 1024        # moe intermediate
FSH = 2048      # shared intermediate (full)
FSHL = FSH // NCORES  # 256 per core
B, S = 2, 2048
T = B * S       # 4096 tokens
P = 128
KH = H // P     # 16
KF = F // P     # 8
KFS = FSHL // P  # 2
NHS = 4         # H-slices of 512
HS = H // NHS   # 512
TCS = 512       # shared-phase token chunk
NCS = T // TCS  # 8 chunks
TOWN = T // NCORES  # 512 owned tokens per core

_CACHED = {}


def _build(CAP):
    CAPB = CAP // P           # slot blocks per expert
    # gate/up psum column chunks: 384 keeps the matmul (not LDWEIGHTS) the
    # pacing op; PSUM bank holds at most 512 fp32 columns.
    CHW = 384 if CAP % 384 == 0 else 512
    TSL = []
    r = CAP
    while r > 0:
        TSL.append(min(CHW, r))
        r -= min(CHW, r)

    nc = bacc.Bacc("TRN2", target_bir_lowering=False, debug=False, num_devices=NCORES)

    hidd_in = nc.dram_tensor("hidd", [H, T], BF16, kind="ExternalInput")
    hidg_in = nc.dram_tensor("hidg", [EL, KH, P, CAP], BF16, kind="ExternalInput")
    gw_in = nc.dram_tensor("gw", [EL, KF, P, KH, P], BF16, kind="ExternalInput")
    uw_in = nc.dram_tensor("uw", [EL, KF, P, KH, P], BF16, kind="ExternalInput")
    dwt_in = nc.dram_tensor("dwt", [EL, KF, P, H], BF16, kind="ExternalInput")
    shg_in = nc.dram_tensor("shg", [KFS, P, KH, P], BF16, kind="ExternalInput")
    shu_in = nc.dram_tensor("shu", [KFS, P, KH, P], BF16, kind="ExternalInput")
    shdt_in = nc.dram_tensor("shdt", [KFS, P, H], BF16, kind="ExternalInput")
    gat_in = nc.dram_tensor("gat", [EL, P, CAPB], F32, kind="ExternalInput")
    sidx_in = nc.dram_tensor("sidx", [EL, P, CAP // 16], I16, kind="ExternalInput")
    out_ext = nc.dram_tensor("out", [TOWN, H], BF16, kind="ExternalOutput")

    with tile.TileContext(nc) as tc:
        with (
            tc.tile_pool(name="shconst", bufs=1) as shconst,
            tc.tile_pool(name="hidsp", bufs=1) as hidsp,
            tc.tile_pool(name="hidgp", bufs=1) as hidgp,
            tc.tile_pool(name="wgt", bufs=2) as wgt,
            tc.tile_pool(name="dwp", bufs=1) as dwp,
            tc.tile_pool(name="silp", bufs=2) as silp,
            tc.tile_pool(name="actp", bufs=1) as actp,
            tc.tile_pool(name="stgp", bufs=1) as stgp,
            tc.tile_pool(name="outp", bufs=4) as outp,
            tc.tile_pool(name="smallp", bufs=1) as smallp,
            tc.tile_pool(name="ps_gu", bufs=2, space="PSUM") as ps_gu,
            tc.tile_pool(name="ps_d", bufs=4, space="PSUM") as ps_d,
            tc.tile_pool(name="dram", bufs=1, space="DRAM") as dram,
        ):
            # DRAM partials, one per 512-wide H-slice
            parts = [dram.tile([T, HS], BF16, name=f"part{h}", tag=f"part{h}")
                     for h in range(NHS)]
            rsouts = [dram.tile([TOWN, HS], BF16, name=f"rsout{h}", tag=f"rsout{h}")
                      for h in range(NHS)]

            # small per-expert constants
            gats = []
            sidxs = []
            for e in range(EL):
                g = smallp.tile([P, CAPB], F32, tag=f"gat{e}")
                nc.sync.dma_start(g[:], gat_in.ap()[e])
                gats.append(g)
                si = smallp.tile([P, CAP // 16], I16, tag=f"sidx{e}")
                nc.sync.dma_start(si[:], sidx_in.ap()[e])
                sidxs.append(si)

            # shared expert weights (resident, small)
            shgs, shus, shds = [], [], []
            for f in range(KFS):
                g = shconst.tile([P, KH * P], BF16, tag=f"shg{f}")
                nc.sync.dma_start(g[:], shg_in.ap()[f].rearrange("p k j -> p (k j)"))
                shgs.append(g)
                u = shconst.tile([P, KH * P], BF16, tag=f"shu{f}")
                nc.sync.dma_start(u[:], shu_in.ap()[f].rearrange("p k j -> p (k j)"))
                shus.append(u)
                d = shconst.tile([P, H], BF16, tag=f"shd{f}")
                nc.sync.dma_start(d[:], shdt_in.ap()[f])
                shds.append(d)

            # ---------------- Phase S: shared expert over all tokens ----------
            for c in range(NCS):
                sl_t = slice(c * TCS, (c + 1) * TCS)
                hidt = []
                for k in range(KH):
                    t_ = hidsp.tile([P, TCS], BF16, tag=f"h{k}", name=f"hs_{c}_{k}")
                    nc.sync.dma_start(t_[:], hidd_in.ap()[k * P:(k + 1) * P, sl_t])
                    hidt.append(t_)
                sacts = []
                for f in range(KFS):
                    gp = ps_gu.tile([P, TCS], F32, tag="gp", name=f"sgp_{c}_{f}")
                    up = ps_gu.tile([P, TCS], F32, tag="up", name=f"sup_{c}_{f}")
                    for k in range(KH):
                        nc.tensor.matmul(
                            gp[:], shgs[f][:, k * P:(k + 1) * P], hidt[k][:],
                            start=(k == 0), stop=(k == KH - 1),
                        )
                    for k in range(KH):
                        nc.tensor.matmul(
                            up[:], shus[f][:, k * P:(k + 1) * P], hidt[k][:],
                            start=(k == 0), stop=(k == KH - 1),
                        )
                    sil = silp.tile([P, TCS], F32, tag="sil", name=f"ssil_{c}_{f}")
                    nc.scalar.activation(sil[:], gp[:], mybir.ActivationFunctionType.Silu)
                    at = actp.tile([P, TCS], BF16, tag=f"sact{f}", name=f"sact_{c}_{f}")
                    nc.vector.tensor_tensor(at[:], sil[:], up[:], mybir.AluOpType.mult)
                    sacts.append(at)
                # token-major down: out[tok, H]
                for tb in range(TCS // P):
                    dps = []
                    for h in range(NHS):
                        dp = ps_d.tile([P, HS], F32, tag="dp", name=f"sdp_{c}_{tb}_{h}")
                        dps.append(dp)
                    for f in range(KFS):
                        for h in range(NHS):
                            nc.tensor.matmul(
                                dps[h][:],
                                sacts[f][:, tb * P:(tb + 1) * P],
                                shds[f][:, h * HS:(h + 1) * HS],
                                start=(f == 0), stop=(f == KFS - 1),
                            )
                    for h in range(NHS):
                        ob = outp.tile([P, HS], BF16, tag="sob", name=f"sob_{c}_{tb}_{h}")
                        nc.vector.tensor_copy(ob[:], dps[h][:])
                        nc.sync.dma_start(
                            parts[h][c * TCS + tb * P: c * TCS + (tb + 1) * P, :],
                            ob[:],
                        )

            # ---------------- Phase R: routed experts -------------------------
            def load_hidg(e):
                tiles = []
                for k in range(KH):
                    t_ = hidgp.tile([P, CAP], BF16, tag=f"hg{k}", name=f"hg_{e}_{k}")
                    nc.sync.dma_start(t_[:], hidg_in.ap()[e, k])
                    tiles.append(t_)
                return tiles

            def load_dwt(e):
                tiles = []
                for kf in range(KF):
                    t_ = dwp.tile([P, H], BF16, tag=f"dw{kf}", name=f"dwt_{e}_{kf}")
                    nc.sync.dma_start(t_[:], dwt_in.ap()[e, kf])
                    tiles.append(t_)
                return tiles

            hidg_e = load_hidg(0)
            dwt_e = load_dwt(0)

            for e in range(EL):
                # ---- gate/up/act over CAP gathered tokens ----
                acts = []
                for f in range(KF):
                    gt = wgt.tile([P, KH * P], BF16, tag="gt", name=f"gt_{e}_{f}")
                    nc.sync.dma_start(gt[:], gw_in.ap()[e, f].rearrange("p k j -> p (k j)"))
                    ut = wgt.tile([P, KH * P], BF16, tag="ut", name=f"ut_{e}_{f}")
                    nc.sync.dma_start(ut[:], uw_in.ap()[e, f].rearrange("p k j -> p (k j)"))
                    at = actp.tile([P, CAP], BF16, tag=f"act{f}", name=f"act_{e}_{f}")
                    col = 0
                    for ts in TSL:
                        sl_s = slice(col, col + ts)
                        gp = ps_gu.tile([P, ts], F32, tag="gp", name=f"gp_{e}_{f}_{col}")
                        up = ps_gu.tile([P, ts], F32, tag="up", name=f"up_{e}_{f}_{col}")
                        for k in range(KH):
                            nc.tensor.matmul(
                                gp[:], gt[:, k * P:(k + 1) * P], hidg_e[k][:, sl_s],
                                start=(k == 0), stop=(k == KH - 1),
                            )
                        for k in range(KH):
                            nc.tensor.matmul(
                                up[:], ut[:, k * P:(k + 1) * P], hidg_e[k][:, sl_s],
                                start=(k == 0), stop=(k == KH - 1),
                            )
                        sil = silp.tile([P, ts], F32, tag="sil", name=f"sil_{e}_{f}_{col}")
                        nc.scalar.activation(sil[:], gp[:], mybir.ActivationFunctionType.Silu)
                        nc.vector.tensor_tensor(at[:, sl_s], sil[:], up[:], mybir.AluOpType.mult)
                        col += ts
                    acts.append(at)

                # prefetch next expert's gathered hidden (reuses same buffers,
                # waits until this expert's gate/up has read them)
                if e + 1 < EL:
                    hidg_next = load_hidg(e + 1)

                # ---- token-major down, scaled by gating, scatter-add ----
                stgs = [stgp.tile([P, CAPB * HS], BF16, tag=f"stg{h}", name=f"stg_{e}_{h}")
                        for h in range(NHS)]
                if True:
                    # hs-outer: finish H-slices early so the serial gpsimd
                    # scatter chain (and, for e==1, the ReduceScatters) start
                    # as soon as possible
                    for h in range(NHS):
                        for b in range(CAPB):
                            dp = ps_d.tile([P, HS], F32, tag="dp", name=f"dp_{e}_{b}_{h}")
                            for kf in range(KF):
                                nc.tensor.matmul(
                                    dp[:],
                                    acts[kf][:, b * P:(b + 1) * P],
                                    dwt_e[kf][:, h * HS:(h + 1) * HS],
                                    start=(kf == 0), stop=(kf == KF - 1),
                                )
                            nc.vector.tensor_tensor(
                                stgs[h][:, b * HS:(b + 1) * HS], dp[:],
                                gats[e][:, b:b + 1].to_broadcast([P, HS]),
                                mybir.AluOpType.mult,
                            )
                        nc.gpsimd.dma_scatter_add(
                            parts[h][:, :],
                            stgs[h][:].rearrange("p (b j) -> p b j", j=HS),
                            sidxs[e][:],
                            num_idxs=CAP,
                            num_idxs_reg=CAP,
                            elem_size=HS,
                        )
                        if e == EL - 1:
                            # ReduceScatter this H-slice (both experts done)
                            nc.gpsimd.collective_compute(
                                "ReduceScatter",
                                mybir.AluOpType.add,
                                replica_groups=[list(range(NCORES))],
                                ins=[parts[h].opt()],
                                outs=[rsouts[h].opt()],
                            )
                            # owned rows out (bf16; host converts to fp32)
                            nc.sync.dma_start(
                                out_ext.ap()[:, h * HS:(h + 1) * HS], rsouts[h][:]
                            )

                if e + 1 < EL:
                    hidg_e = hidg_next
                    dwt_e = load_dwt(e + 1)

    nc.compile()
    return nc


def _route_host(hidden, gate_weight, bias):
    """Exact replication of the reference routing in float64."""
    logits = hidden.astype(np.float64) @ gate_weight.astype(np.float64).T
    scores = 1.0 / (1.0 + np.exp(-logits))                  # [T, E]
    sfc = scores + bias.astype(np.float64)[None, :]
    Tn = scores.shape[0]
    grp = sfc.reshape(Tn, G, EPG)
    top2 = np.sort(grp, axis=-1)[:, :, -2:].sum(-1)         # [T, G]
    gidx = np.argsort(-top2, axis=1, kind="stable")[:, :2]
    gmask = np.zeros((Tn, G), bool)
    np.put_along_axis(gmask, gidx, True, 1)
    smask = np.repeat(gmask, EPG, axis=1)
    masked = np.where(smask, sfc, -np.inf)
    tidx = np.argsort(-masked, axis=1, kind="stable")[:, :TOPK]   # [T, K]
    tw = np.take_along_axis(scores, tidx, axis=1)           # raw sigmoid scores
    tw = tw / (tw.sum(-1, keepdims=True) + 1e-20) * SCALE
    return tw.astype(np.float32), tidx


def _prep_core_inputs(core, hidbf, gate_w, up_w, down_w, sh_gate_w, sh_up_w,
                      sh_down_w, slot_tok, slot_gat, CAP):
    import ml_dtypes
    bf = ml_dtypes.bfloat16
    e0 = 2 * core

    def tile_kxm(w):  # w [F', H] -> [KF', P, KH, P]; lhsT[p(H), j(F)] per (f,k)
        Fp = w.shape[0]
        return np.ascontiguousarray(w.reshape(Fp // P, P, KH, P).transpose(0, 3, 2, 1))

    gw = np.stack([tile_kxm(gate_w[e0 + e]) for e in range(EL)])
    uw = np.stack([tile_kxm(up_w[e0 + e]) for e in range(EL)])
    # down token-major: dwt[e, kf, p, h] = down_w[e][h, kf*128+p]
    dwt = np.stack([np.ascontiguousarray(down_w[e0 + e].T.reshape(KF, P, H))
                    for e in range(EL)])

    sl = slice(core * FSHL, (core + 1) * FSHL)
    shg = tile_kxm(sh_gate_w[sl])
    shu = tile_kxm(sh_up_w[sl])
    shdt = np.ascontiguousarray(sh_down_w[:, sl].T.reshape(KFS, P, H))

    # gathered hidden, gating, scatter indices for the 2 local experts
    hidg = np.empty((EL, KH, P, CAP), dtype=bf)
    gat = np.zeros((EL, P, CAP // P), dtype=np.float32)
    sidx = np.zeros((EL, P, CAP // 16), dtype=np.int16)
    for e in range(EL):
        toks = slot_tok[e0 + e]
        gats = slot_gat[e0 + e]
        hidg[e] = hidbf[:, toks].reshape(KH, P, CAP)
        gat[e] = gats.reshape(CAP // P, P).T
        sidx[e] = np.tile(toks.astype(np.int16).reshape(CAP // 16, 16).T, (8, 1))

    return {
        "hidd": hidbf, "hidg": hidg,
        "gw": gw.astype(bf), "uw": uw.astype(bf), "dwt": dwt.astype(bf),
        "shg": shg.astype(bf), "shu": shu.astype(bf), "shdt": shdt.astype(bf),
        "gat": gat, "sidx": sidx,
    }


def kernel(hidden_states, gate_weight, e_score_correction_bias,
           gate_w, up_w, down_w, sh_gate_w, sh_up_w, sh_down_w):
    import ml_dtypes
    bf = ml_dtypes.bfloat16
    hidden_states = np.asarray(hidden_states, dtype=np.float32)
    gate_weight = np.asarray(gate_weight, dtype=np.float32)
    bias = np.asarray(e_score_correction_bias, dtype=np.float32)
    gate_w = np.asarray(gate_w, dtype=np.float32)
    up_w = np.asarray(up_w, dtype=np.float32)
    down_w = np.asarray(down_w, dtype=np.float32)
    sh_gate_w = np.asarray(sh_gate_w, dtype=np.float32)
    sh_up_w = np.asarray(sh_up_w, dtype=np.float32)
    sh_down_w = np.asarray(sh_down_w, dtype=np.float32)

    hidden = hidden_states.reshape(T, H)
    tw, tidx = _route_host(hidden, gate_weight, bias)

    # per-expert slot lists (sorted, padded with token 0 / gating 0)
    counts = np.bincount(tidx.ravel(), minlength=E)
    CAP = max(1152, int(np.ceil(counts.max() / P)) * P)
    slot_tok, slot_gat = [], []
    for e in range(E):
        rows, cols = np.nonzero(tidx == e)
        order = np.argsort(rows, kind="stable")
        toks = rows[order].astype(np.int64)
        gats = tw[rows[order], cols[order]]
        pad = CAP - len(toks)
        # pad slots must carry UNIQUE indices within the scatter call (gating 0
        # makes their contribution exactly +0.0): duplicate rows in one
        # dma_scatter_add race on the read-modify-write and can drop real adds.
        unused = np.setdiff1d(np.arange(T, dtype=np.int64), toks)[:pad]
        assert len(unused) == pad, (len(unused), pad)
        slot_tok.append(np.concatenate([toks, unused]))
        slot_gat.append(np.concatenate([gats, np.zeros(pad, np.float32)]).astype(np.float32))

    if ("nc", CAP) not in _CACHED:
        _CACHED[("nc", CAP)] = _build(CAP)
    nc = _CACHED[("nc", CAP)]

    hidbf = np.ascontiguousarray(hidden.T).astype(bf)   # [H, T]
    in_maps = [
        _prep_core_inputs(c, hidbf, gate_w, up_w, down_w, sh_gate_w, sh_up_w,
                          sh_down_w, slot_tok, slot_gat, CAP)
        for c in range(NCORES)
    ]
    res = run_bass_kernel_spmd(nc, in_maps, core_ids=list(range(NCORES)))
    _CACHED["last_res"] = res
    out = np.concatenate([res.results[c]["out"].astype(np.float32) for c in range(NCORES)], axis=0)
    return out.reshape(B, S, H).astype(np.float32)
